# revision 11
# baseline (speedup 1.0000x reference)
"""Causal self-attention block (QKV proj + causal MHA + out proj + residual
+ LayerNorm) for B=4, S=2048, HID=1024, 16 heads, on 8 Trainium2 cores.

Sharding: core c handles batch b=c//2 and heads [8h, 8h+8) where h=c%2
(Megatron-style head split within a batch pair). Each core computes its 8
heads' attention for all 2048 rows. The out-projection is row-split: for
each 512-query tile, the pair exchanges normalized attention outputs via
a pairwise AllToAll (core sends its heads' columns for the partner's 256
rows), after which each core holds all 16 heads for its own 256 rows and
does the full out-projection + residual + LayerNorm locally — no
partial-sum ReduceScatter and no epilogue after the last tile beyond one
small exchange.

v3 schedule: all matmul operands are bf16 (PSUM stays fp32), enabling
fast weight load and halving DMA. The attention inner loop is
software-pipelined (QK two key-blocks ahead of PV), and the scalar-engine
exp is the rate limiter during attention, so projection matmuls for tile
t+1 and out-projection matmuls for tile t-2 are interleaved INTO the
attention block stream (the PE queue is FIFO; fillers are emitted between
blocks so the PE fills exp-wait gaps). Causal masking runs on the GpSimd
engine to keep the Vector queue clear of the softmax-normalize chain.
"""

from collections import deque

import numpy as np
import ml_dtypes

import concourse.bacc as bacc
import concourse.mybir as mybir
import concourse.tile as tile
from concourse.bass import broadcast_tensor_aps
from concourse.bass_utils import run_bass_kernel_spmd

F32 = mybir.dt.float32
BF16 = mybir.dt.bfloat16
AF = mybir.ActivationFunctionType
OP = mybir.AluOpType

N_CORES = 8
B, S, HID = 4, 2048, 1024
NHC = 8          # heads per core
DH = 64          # head dim
HW = 512         # per-core head width (NHC * DH)
SQT = 512        # sq tile width
NSQT = S // SQT  # 4
NHCH = HID // 128  # 8 hid chunks
OWN = 256        # rows per core per sq tile (row-split epilogue)
EPS = 1e-5

_CACHE = {}


def _build():
    nc = bacc.Bacc("TRN2", target_bir_lowering=False, debug=False,
                   num_devices=N_CORES)

    xT = nc.dram_tensor("xT", [HID, S], BF16, kind="ExternalInput").ap()
    xh = nc.dram_tensor("xh", [4 * OWN, HID], F32, kind="ExternalInput").ap()
    wqT = nc.dram_tensor("wqT", [HID, HW], BF16, kind="ExternalInput").ap()
    wkT = nc.dram_tensor("wkT", [HID, HW], BF16, kind="ExternalInput").ap()
    wvT = nc.dram_tensor("wvT", [HID, HW], BF16, kind="ExternalInput").ap()
    woT = nc.dram_tensor("woT", [HID, HID], BF16, kind="ExternalInput").ap()
    bq4 = nc.dram_tensor("bq4", [128, 4], F32, kind="ExternalInput").ap()
    bk4 = nc.dram_tensor("bk4", [128, 4], F32, kind="ExternalInput").ap()
    bvb = nc.dram_tensor("bvb", [128, HW], F32, kind="ExternalInput").ap()
    gmb = nc.dram_tensor("gmb", [128, HID], F32, kind="ExternalInput").ap()
    btb = nc.dram_tensor("btb", [128, HID], F32, kind="ExternalInput").ap()
    m128 = nc.dram_tensor("m128", [128, 128], BF16, kind="ExternalInput").ap()
    vone = nc.dram_tensor("vone", [128, 8], BF16, kind="ExternalInput").ap()

    sel = nc.dram_tensor("sel", [128, 2], F32, kind="ExternalInput").ap()

    out = nc.dram_tensor("out", [4 * OWN, HID], F32, kind="ExternalOutput").ap()

    # pairwise exchange buffers: shard g rows [256g:256g+256] hold this
    # core's (sel-masked) contribution to rank g's rows, slab s = heads of
    # pair-rank s.  RS-add concatenates the two cores' head-halves.
    rs_i = [nc.dram_tensor(f"rs_i{t}", [512, HID], BF16).ap()
            for t in range(NSQT)]
    rs_o = [nc.dram_tensor(f"rs_o{t}", [256, HID], BF16).ap()
            for t in range(NSQT)]
    GROUPS = [[0, 1], [2, 3], [4, 5], [6, 7]]

    from contextlib import ExitStack
    with tile.TileContext(nc) as tc, ExitStack() as es:
        TP = tc.tile_pool
        cp = es.enter_context(TP(name="consts", bufs=1))
        ktp = es.enter_context(TP(name="kt", bufs=1))
        vtp = es.enter_context(TP(name="vt", bufs=1))
        wop = es.enter_context(TP(name="wo", bufs=1))
        ep = es.enter_context(TP(name="exp", bufs=2))
        avp = es.enter_context(TP(name="av", bufs=2))
        rp = es.enter_context(TP(name="rcp", bufs=2))
        sxp = es.enter_context(TP(name="sx", bufs=1))
        rvp = es.enter_context(TP(name="rv", bufs=1))
        pp = es.enter_context(TP(name="pp", bufs=2, space="PSUM"))
        sp = es.enter_context(TP(name="sp", bufs=2, space="PSUM"))
        app = es.enter_context(TP(name="ap", bufs=1, space="PSUM"))
        wp = es.enter_context(TP(name="wqkv", bufs=1))
        xp = es.enter_context(TP(name="xts", bufs=1))
        qtp = es.enter_context(TP(name="qt", bufs=1))
        lp = es.enter_context(TP(name="ln", bufs=2))
        lsp = es.enter_context(TP(name="lns", bufs=2))

        # ---- constants + weights, spread across the three DMA queues ----
        wq = [wp.tile([128, HW], BF16, name=f"wq{hh}") for hh in range(NHCH)]
        wk = [wp.tile([128, HW], BF16, name=f"wk{hh}") for hh in range(NHCH)]
        wv = [wp.tile([128, HW], BF16, name=f"wv{hh}") for hh in range(NHCH)]
        xts2 = [[xp.tile([128, SQT], BF16, name=f"xt{par}_{hh}")
                 for hh in range(NHCH)] for par in range(2)]
        qts2 = [[qtp.tile([128, SQT], BF16, name=f"q{par}_{m}")
                 for m in range(4)] for par in range(2)]

        for hh in range(NHCH):
            nc.sync.dma_start(wq[hh][:], wqT[128 * hh:128 * (hh + 1), :])
        for hh in range(NHCH):
            nc.scalar.dma_start(xts2[0][hh][:],
                                xT[128 * hh:128 * (hh + 1), 0:SQT])
        for hh in range(NHCH):
            nc.scalar.dma_start(wk[hh][:], wkT[128 * hh:128 * (hh + 1), :])
        for hh in range(NHCH):
            nc.scalar.dma_start(wv[hh][:], wvT[128 * hh:128 * (hh + 1), :])

        bqs = cp.tile([128, 4], F32)
        nc.gpsimd.dma_start(bqs[:], bq4[:])
        bks = cp.tile([128, 4], F32)
        nc.gpsimd.dma_start(bks[:], bk4[:])
        bvs = cp.tile([128, HW], F32)
        nc.gpsimd.dma_start(bvs[:], bvb[:])
        vos = cp.tile([128, 8], BF16)
        nc.gpsimd.dma_start(vos[:], vone[:])
        mask = cp.tile([128, 128], BF16)
        nc.gpsimd.dma_start(mask[:], m128[:])
        wot = [wop.tile([128, HID], BF16, name=f"wo{d}") for d in range(NHCH)]
        for d in range(NHCH):
            nc.sync.dma_start(wot[d][:], woT[128 * d:128 * (d + 1), :])
        gms = cp.tile([128, HID], F32)
        nc.sync.dma_start(gms[:], gmb[:])
        bts = cp.tile([128, HID], F32)
        nc.sync.dma_start(bts[:], btb[:])
        sels = cp.tile([128, 2], F32)
        nc.gpsimd.dma_start(sels[:], sel[:])
        magicc = cp.tile([128, 1], mybir.dt.uint32)
        nc.vector.memset(magicc[:], 0x5f3759df)
        mhalf = cp.tile([128, 1], F32)
        nc.vector.memset(mhalf[:], -0.5)

        kt = [ktp.tile([128, S], BF16, name=f"kt{p}") for p in range(4)]
        vt = [vtp.tile([128, 8, 65], BF16, name=f"vt{i}") for i in range(16)]
        sxs = [sxp.tile([128, 2, 4, SQT], BF16, name=f"sx{par}")
               for par in range(2)]
        rvs = [rvp.tile([128, 2, 4, OWN], BF16, name=f"rv{t}")
               for t in range(NSQT)]

        # ---------- QKV projection emission ----------
        def emit_A0():
            """First tile's projections, hh-outer 2-pass for early start."""
            xts, qts = xts2[0], qts2[0]
            for wgt, evac in (
                (wq, lambda m, ps: nc.vector.tensor_scalar_add(
                    qts[m][:], ps[:], bqs[:, m:m + 1])),
                (wk, lambda m, ps: nc.vector.tensor_scalar_add(
                    kt[m][:, 0:SQT], ps[:], bks[:, m:m + 1])),
            ):
                for mp in range(2):
                    pss = [pp.tile([128, SQT], F32, tag="pq", name=f"pqa{u}")
                           for u in range(2)]
                    for hh in range(NHCH):
                        for u in range(2):
                            m = 2 * mp + u
                            nc.tensor.matmul(
                                pss[u][:], wgt[hh][:, 128 * m:128 * (m + 1)],
                                xts[hh][:], start=(hh == 0),
                                stop=(hh == NHCH - 1))
                    for u in range(2):
                        evac(2 * mp + u, pss[u])
            for sp_ in range(2):
                pss = [pp.tile([128, HW], F32, tag="pq", name=f"pqv{u}")
                       for u in range(2)]
                for hh in range(NHCH):
                    for u in range(2):
                        s_ = 2 * sp_ + u
                        nc.tensor.matmul(
                            pss[u][:], xts[hh][:, 128 * s_:128 * (s_ + 1)],
                            wv[hh][:], start=(hh == 0), stop=(hh == NHCH - 1))
                for u in range(2):
                    i = 2 * sp_ + u
                    nc.vector.tensor_tensor(
                        vt[i][:, :, 0:64], pss[u][:], bvs[:], op=OP.add)
                    nc.vector.tensor_copy(vt[i][:, :, 64:65], vos[:])

        def dma_x(t):
            xts = xts2[t % 2]
            for hh in range(NHCH):
                nc.sync.dma_start(xts[hh][:], xT[128 * hh:128 * (hh + 1),
                                                 SQT * t:SQT * (t + 1)])

        def groups_A(t):
            """Projection filler groups for tile t (8 matmuls each)."""
            xts, qts = xts2[t % 2], qts2[t % 2]

            def qg(m):
                def f():
                    ps = pp.tile([128, SQT], F32, tag="pq")
                    for hh in range(NHCH):
                        nc.tensor.matmul(
                            ps[:], wq[hh][:, 128 * m:128 * (m + 1)], xts[hh][:],
                            start=(hh == 0), stop=(hh == NHCH - 1))
                    nc.vector.tensor_scalar_add(qts[m][:], ps[:],
                                                bqs[:, m:m + 1])
                return f

            def kg(m):
                def f():
                    ps = pp.tile([128, SQT], F32, tag="pq")
                    for hh in range(NHCH):
                        nc.tensor.matmul(
                            ps[:], wk[hh][:, 128 * m:128 * (m + 1)], xts[hh][:],
                            start=(hh == 0), stop=(hh == NHCH - 1))
                    nc.vector.tensor_scalar_add(
                        kt[m][:, SQT * t:SQT * (t + 1)], ps[:], bks[:, m:m + 1])
                return f

            def vg(s_):
                def f():
                    i = 4 * t + s_
                    ps = pp.tile([128, HW], F32, tag="pq")
                    for hh in range(NHCH):
                        nc.tensor.matmul(
                            ps[:], xts[hh][:, 128 * s_:128 * (s_ + 1)], wv[hh][:],
                            start=(hh == 0), stop=(hh == NHCH - 1))
                    nc.vector.tensor_tensor(
                        vt[i][:, :, 0:64], ps[:], bvs[:], op=OP.add)
                    nc.vector.tensor_copy(vt[i][:, :, 64:65], vos[:])
                return f

            return ([qg(m) for m in range(4)] + [kg(m) for m in range(4)]
                    + [vg(s_) for s_ in range(4)])

        # ---------- attention ----------
        def emit_QK(p, i, j, qts):
            d = i - 4 * j
            lo = 128 * d if d >= 0 else 0
            s2 = sp.tile([128, 2 * SQT], F32, tag="s2")
            nc.tensor.matmul(
                s2[:, lo:SQT],
                kt[p][0:64, 128 * i:128 * (i + 1)],
                qts[p][0:64, lo:SQT],
                start=True, stop=True, tile_position=(0, 0))
            nc.tensor.matmul(
                s2[:, SQT + lo:2 * SQT],
                kt[p][64:128, 128 * i:128 * (i + 1)],
                qts[p][64:128, lo:SQT],
                start=True, stop=True, tile_position=(64, 0))
            return s2

        def emit_exp(p, i, j, s2):
            d = i - 4 * j
            lo = 128 * d if d >= 0 else 0
            e2 = ep.tile([128, 2 * SQT], BF16, tag="e2")
            s2v = s2[:].rearrange("p (a b) -> p a b", a=2)
            e2v = e2[:].rearrange("p (a b) -> p a b", a=2)
            nc.scalar.activation(e2v[:, :, lo:SQT], s2v[:, :, lo:SQT],
                                 AF.Exp, scale=0.125)
            if d >= 0:
                ea = e2v[:, :, lo:lo + 128]
                ma = mask[:].rearrange("p (a b) -> p a b", a=1)
                ea2, ma2 = broadcast_tensor_aps(ea, ma)
                nc.gpsimd.tensor_tensor(ea2, ea2, ma2, op=OP.mult)
            return e2

        def emit_PV(p, i, j, e2, pv2):
            d = i - 4 * j
            lo = 128 * d if d >= 0 else 0
            nc.tensor.matmul(
                pv2[0:65, lo:SQT], vt[i][:, 2 * p, :], e2[:, lo:SQT],
                start=(i == 0), stop=(i == 4 * j + 3))
            nc.tensor.matmul(
                pv2[0:65, SQT + lo:2 * SQT], vt[i][:, 2 * p + 1, :],
                e2[:, SQT + lo:2 * SQT],
                start=(i == 0), stop=(i == 4 * j + 3))

        def emit_B(t, fillers, fill_every):
            ni = 4 * t + 4
            qts = qts2[t % 2]
            sx = sxs[t % 2]
            nblk = 0
            for p in range(4):
                pv2 = app.tile([128, 2 * SQT], F32, tag="pv2")
                s2s = {0: emit_QK(p, 0, t, qts)}
                if ni > 1:
                    s2s[1] = emit_QK(p, 1, t, qts)
                for i in range(ni):
                    e2 = emit_exp(p, i, t, s2s.pop(i))
                    emit_PV(p, i, t, e2, pv2)
                    if i + 2 < ni:
                        s2s[i + 2] = emit_QK(p, i + 2, t, qts)
                    nblk += 1
                    if nblk % fill_every == 0 and fillers:
                        fillers.popleft()()
                # normalize: evac PSUM, recip of sums row, broadcast, then
                # sel-masked scale into the two exchange slabs (the slab of
                # the partner's heads gets zeros on this core)
                av2 = avp.tile([65, 2 * SQT], F32, tag="av")
                nc.vector.tensor_copy(av2[:], pv2[0:65, :])
                sm = rp.tile([1, 2 * SQT], F32, tag="sm")
                nc.vector.tensor_copy(sm[:], av2[64:65, :])
                rc = rp.tile([1, 2 * SQT], F32, tag="rc")
                nc.vector.reciprocal_approx_fast(rc[:], sm[:])
                rb = rp.tile([64, 2 * SQT], F32, tag="rb")
                nc.gpsimd.partition_broadcast(rb[:], rc[:])
                for hb in range(2):
                    for sh in range(2):
                        nc.vector.scalar_tensor_tensor(
                            sx[64 * hb:64 * (hb + 1), sh, p, :],
                            av2[0:64, SQT * hb:SQT * (hb + 1)],
                            sels[0:64, sh:sh + 1],
                            rb[:, SQT * hb:SQT * (hb + 1)],
                            op0=OP.mult, op1=OP.mult)
            # pairwise exchange: RS-add of sel-masked slabs
            for g in range(2):
                for sh in range(2):
                    nc.sync.dma_start(
                        rs_i[t][256 * g + 128 * sh:256 * g + 128 * (sh + 1), :],
                        sx[:, sh, :, OWN * g:OWN * (g + 1)])
            nc.gpsimd.collective_compute(
                "ReduceScatter", OP.add, replica_groups=GROUPS,
                ins=[rs_i[t][:]], outs=[rs_o[t][:]])

        def emit_recv(t):
            for sh in range(2):
                nc.sync.dma_start(rvs[t][:, sh, :, :],
                                  rs_o[t][128 * sh:128 * (sh + 1), :])

        # ---------- out projection + residual + LayerNorm (own rows) ----
        def groups_C(t):
            rv = rvs[t]
            xcs = [lp.tile([128, HID], F32, tag=f"xc{c_}", name=f"xc{c_}")
                   for c_ in range(2)]
            xhs = [lp.tile([128, HID], F32, tag=f"xh{c_}", name=f"xhs{c_}")
                   for c_ in range(2)]

            def cg(c_, o):
                def f():
                    if o == 0:
                        nc.sync.dma_start(
                            xhs[c_][:],
                            xh[OWN * t + 128 * c_:OWN * t + 128 * (c_ + 1), :])
                    po = pp.tile([128, SQT], F32, tag="pq")
                    for sh in range(2):
                        for tt in range(4):
                            nc.tensor.matmul(
                                po[:],
                                rv[:, sh, tt, 128 * c_:128 * (c_ + 1)],
                                wot[4 * sh + tt][:, SQT * o:SQT * (o + 1)],
                                start=(sh == 0 and tt == 0),
                                stop=(sh == 1 and tt == 3))
                    nc.vector.tensor_add(
                        xcs[c_][:, SQT * o:SQT * (o + 1)], po[:],
                        xhs[c_][:, SQT * o:SQT * (o + 1)])
                return f

            def lnf(c_):
                def f():
                    xc = xcs[c_]
                    st6 = lsp.tile([128, 12], F32, tag="st6")
                    nc.vector.bn_stats(st6[:, 0:6], xc[:, 0:512])
                    nc.vector.bn_stats(st6[:, 6:12], xc[:, 512:1024])
                    mv = lsp.tile([128, 2], F32, tag="mv")
                    nc.vector.bn_aggr(mv[:], st6[:])
                    # 1/sigma via integer fast-rsqrt seed + 2 Newton steps,
                    # entirely on the vector engine.
                    ve = lsp.tile([128, 1], F32, tag="ve")
                    nc.vector.tensor_scalar_add(ve[:], mv[:, 1:2], EPS)
                    inv = lsp.tile([128, 1], F32, tag="inv")
                    nc.vector.tensor_scalar(
                        inv[:].bitcast(mybir.dt.uint32),
                        ve[:].bitcast(mybir.dt.uint32),
                        1, None, op0=OP.logical_shift_right)
                    nc.vector.tensor_tensor(
                        inv[:].bitcast(mybir.dt.uint32), magicc[:],
                        inv[:].bitcast(mybir.dt.uint32), op=OP.subtract)
                    nt = lsp.tile([128, 1], F32, tag="nt")
                    for _ in range(2):
                        nc.vector.tensor_mul(nt[:], inv[:], inv[:])
                        nc.vector.scalar_tensor_tensor(
                            nt[:], nt[:], ve[:], mhalf[:],
                            op0=OP.mult, op1=OP.mult)
                        nc.vector.tensor_scalar_add(nt[:], nt[:], 1.5)
                        nc.vector.tensor_mul(inv[:], inv[:], nt[:])
                    nc.vector.scalar_tensor_tensor(
                        xc[:], xc[:], mv[:, 0:1], gms[:],
                        op0=OP.subtract, op1=OP.mult)
                    nc.vector.scalar_tensor_tensor(
                        xc[:], xc[:], inv[:], bts[:],
                        op0=OP.mult, op1=OP.add)
                    nc.sync.dma_start(
                        out[OWN * t + 128 * c_:OWN * t + 128 * (c_ + 1), :],
                        xc[:])
                return f

            return [cg(0, 0), cg(0, 1), lnf(0), cg(1, 0), cg(1, 1), lnf(1)]

        # ---- main schedule ----
        emit_A0()
        fillers = deque()
        FILL_EVERY = {0: 1, 1: 2, 2: 2, 3: 4}
        for t in range(NSQT):
            if t < NSQT - 1:
                dma_x(t + 1)
                fillers.extend(groups_A(t + 1))
            if t == 2:
                fillers.extend(groups_C(0))
            if t == 3:
                fillers.extend(groups_C(1))
                fillers.extend(groups_C(2))
            if t >= 1:
                emit_recv(t - 1)
            emit_B(t, fillers, FILL_EVERY[t])
            while fillers:
                fillers.popleft()()
        emit_recv(3)
        for f in groups_C(3):
            f()

    nc.compile()
    return nc


def _prep_inputs(x, Wq, bq, Wk, bk, Wv, bv, Wo, bo, gamma, beta):
    """Shard + lay out the full inputs for the 8 cores."""
    f32 = np.float32
    bf16 = ml_dtypes.bfloat16
    x = np.asarray(x, f32)
    Wq, bq = np.asarray(Wq, f32), np.asarray(bq, f32)
    Wk, bk = np.asarray(Wk, f32), np.asarray(bk, f32)
    Wv, bv = np.asarray(Wv, f32), np.asarray(bv, f32)
    Wo, bo = np.asarray(Wo, f32), np.asarray(bo, f32)
    gamma, beta = np.asarray(gamma, f32), np.asarray(beta, f32)

    mask = np.triu(np.ones((128, 128), f32)).astype(bf16)
    vone = np.ones((128, 8), bf16)
    gmb = np.ascontiguousarray(np.broadcast_to(gamma, (128, HID)))
    btb = np.ascontiguousarray(np.broadcast_to(beta, (128, HID)))
    woT = np.ascontiguousarray(Wo.T).astype(bf16)

    halves = []
    for h in range(2):
        sl = slice(HW * h, HW * (h + 1))
        halves.append(dict(
            wqT=np.ascontiguousarray(Wq.T[:, sl]).astype(bf16),
            wkT=np.ascontiguousarray(Wk.T[:, sl]).astype(bf16),
            wvT=np.ascontiguousarray(Wv.T[:, sl]).astype(bf16),
            bq4=np.ascontiguousarray(bq[sl].reshape(4, 128).T),
            bk4=np.ascontiguousarray(bk[sl].reshape(4, 128).T),
            bvb=np.ascontiguousarray(np.broadcast_to(bv[sl], (128, HW))),
        ))

    in_maps = []
    for c in range(N_CORES):
        b, h = c // 2, c % 2
        m = dict(halves[h])
        m["xT"] = np.ascontiguousarray(x[b].T).astype(bf16)
        # own rows: per tile t this core owns rows 512t+256h .. +256
        m["xh"] = np.ascontiguousarray(
            np.concatenate([x[b, SQT * t + OWN * h:SQT * t + OWN * h + OWN, :]
                            for t in range(NSQT)], axis=0) + bo)
        m["woT"] = woT
        m["gmb"] = gmb
        m["btb"] = btb
        m["m128"] = mask
        m["vone"] = vone
        selv = np.zeros((128, 2), f32)
        selv[:, h] = 1.0
        m["sel"] = selv
        in_maps.append(m)
    return in_maps


def _run(inputs, trace=False):
    if "nc" not in _CACHE:
        _CACHE["nc"] = _build()
    nc = _CACHE["nc"]
    in_maps = _prep_inputs(**inputs)
    res = run_bass_kernel_spmd(nc, in_maps, list(range(N_CORES)),
                               trace=trace)
    out = np.empty((B, S, HID), np.float32)
    for c in range(N_CORES):
        b, h = c // 2, c % 2
        o = res.results[c]["out"]
        for t in range(NSQT):
            out[b, SQT * t + OWN * h:SQT * t + OWN * h + OWN, :] = \
                o[OWN * t:OWN * (t + 1), :]
    return out, res


def kernel(**inputs):
    out, _ = _run(inputs, trace=False)
    return out
